# revision 1
# baseline (speedup 1.0000x reference)
"""Differentiable Logic Layer kernel for Trainium2 (8 NeuronCores).

Problem: out[t, j] = sum_g softmax(logits)[j, g] * gate_g(a, b)
         with a = x[t, a_idx[j]], b = x[t, b_idx[j]],
         x: [2048, 8192] f32, logits: [16384, 16] f32.

All 16 two-input soft gates are affine in {1, a, b, a*b}, so the mixture
collapses to 4 per-gate coefficients (computed on host from softmax):
    out = w0[j] + wa[j]*a + wb[j]*b + wab[j]*a*b

Sharding: out_dim across the 8 cores (core c owns gates [c*2048, (c+1)*2048)),
x is replicated as xT_f16 [8192, 2048] = (x - 0.5).T (the -0.5 shift halves
fp16 quantization error; coefficients are re-derived for shifted inputs).

Device pipeline, per j-subtile s (128 gates x 2048 batch):
  - indirect row-gather b(s), a(s) from xT (SWDGE, [128,1] int32 offsets)
  - u = b * wab + wa     (DVE tensor_scalar, per-partition fp32 scalars)
  - v = b * wb  + w0     (ACT Identity, per-partition scale/bias)
  - m = u * a            (DVE tensor_tensor, fp16)
  - o = m + v            (DVE tensor_tensor, fp16)
  - store o -> out[s*128:(s+1)*128, :] j-major fp16 (HWDGE)
Host reassembles: result[t, j] = concat_c(out_c.T) upcast to f32 (exact,
values are already fp16).
"""

import numpy as np

import concourse.bacc as bacc
import concourse.bass as bass
import concourse.mybir as mybir
from concourse.bass_utils import run_bass_kernel_spmd

B = 2048
IN_DIM = 8192
OUT_DIM = 16384
N_CORES = 8

F32 = mybir.dt.float32
F16 = mybir.dt.float16
I32 = mybir.dt.int32
MULT = mybir.AluOpType.mult
ADD = mybir.AluOpType.add


def _build_kernel(n_feat=IN_DIM, n_t=B, j_core=OUT_DIM // N_CORES, nbuf=8,
                  sim_safe=False):
    sub = 128
    ns = j_core // sub

    nc = bacc.Bacc("TRN2", target_bir_lowering=False, debug=False)

    xT = nc.dram_tensor("xT", [n_feat, n_t], F16, kind="ExternalInput")
    # meta packs idx (int32, cols [0, 2ns)) and coef (f32 bits, cols [2ns, 6ns))
    meta = nc.dram_tensor("meta", [128, 6 * ns], I32, kind="ExternalInput")
    out = nc.dram_tensor("out", [j_core, n_t], F16, kind="ExternalOutput")

    meta_sb = nc.alloc_sbuf_tensor("meta_sb", [128, 6 * ns], I32)
    idx_sb = meta_sb
    coef_sb = meta_sb.bitcast(F32)
    CO = 2 * ns
    a_sb = [nc.alloc_sbuf_tensor(f"a_sb{i}", [128, n_t], F16) for i in range(nbuf)]
    b_sb = [nc.alloc_sbuf_tensor(f"b_sb{i}", [128, n_t], F16) for i in range(nbuf)]
    v_sb = [nc.alloc_sbuf_tensor(f"v_sb{i}", [128, n_t], F16) for i in range(2)]
    u_sb = nc.alloc_sbuf_tensor("u_sb", [128, n_t], F16)
    m_sb = nc.alloc_sbuf_tensor("m_sb", [128, n_t], F16)
    o_sb = [nc.alloc_sbuf_tensor(f"o_sb{i}", [128, n_t], F16) for i in range(4)]

    load_sem = nc.alloc_semaphore("load_sem")
    ga_sems = [nc.alloc_semaphore(f"ga{s}") for s in range(ns)]
    gb_sems = [nc.alloc_semaphore(f"gb{s}") for s in range(ns)]
    st_sems = [nc.alloc_semaphore(f"st{s}") for s in range(ns)]
    act_sem = nc.alloc_semaphore("act_sem")
    dve_sem = nc.alloc_semaphore("dve_sem")
    u_sem = nc.alloc_semaphore("u_sem")
    m_sem = nc.alloc_semaphore("m_sem")

    out_r = out.rearrange("(s p) t -> s p t", p=128)

    with nc.Block(no_gpsimd_drain=True) as block:

        @block.sync
        def _(sync: bass.BassEngine):
            sync.dma_start(meta_sb[:], meta[:]).then_inc(load_sem, 16)
            for s in range(ns):
                sync.wait_ge(dve_sem, s + 1)
                sync.dma_start(out_r[s], o_sb[s % 4][:]).then_inc(st_sems[s], 16)
            for s in range(ns):
                sync.wait_ge(st_sems[s], 16)

        @block.gpsimd
        def _(gpsimd: bass.BassGpSimd):
            gpsimd.wait_ge(load_sem, 16)
            for s in range(ns):
                if s >= nbuf:
                    gpsimd.wait_ge(dve_sem, s - nbuf + 1)
                    gpsimd.wait_ge(act_sem, s - nbuf + 1)
                gpsimd.indirect_dma_start(
                    out=b_sb[s % nbuf][:], out_offset=None, in_=xT[:],
                    in_offset=bass.IndirectOffsetOnAxis(
                        ap=idx_sb[:, ns + s:ns + s + 1], axis=0),
                ).then_inc(gb_sems[s], 16)
                gpsimd.indirect_dma_start(
                    out=a_sb[s % nbuf][:], out_offset=None, in_=xT[:],
                    in_offset=bass.IndirectOffsetOnAxis(
                        ap=idx_sb[:, s:s + 1], axis=0),
                ).then_inc(ga_sems[s], 16)

        @block.scalar
        def _(scalar: bass.BassScalarEngine):
            scalar.wait_ge(load_sem, 16)
            for s in range(ns):
                scalar.wait_ge(gb_sems[s], 16)
                if s >= 2:
                    scalar.wait_ge(dve_sem, s - 1)
                scalar.activation(
                    v_sb[s % 2][:], b_sb[s % nbuf][:],
                    mybir.ActivationFunctionType.Identity,
                    bias=coef_sb[:, CO + s:CO + s + 1],
                    scale=coef_sb[:, CO + 2 * ns + s:CO + 2 * ns + s + 1],
                ).then_inc(act_sem, 1)

        @block.vector
        def _(vector: bass.BassVectorEngine):
            vector.wait_ge(load_sem, 16)
            for s in range(ns):
                if s >= 4:
                    vector.wait_ge(st_sems[s - 4], 16)
                vector.wait_ge(gb_sems[s], 16)
                if sim_safe and s >= 1:
                    vector.wait_ge(m_sem, s)
                vector.tensor_scalar(
                    u_sb[:], b_sb[s % nbuf][:],
                    coef_sb[:, CO + 3 * ns + s:CO + 3 * ns + s + 1],
                    coef_sb[:, CO + ns + s:CO + ns + s + 1], MULT, ADD,
                ).then_inc(u_sem, 1)
                vector.wait_ge(ga_sems[s], 16)
                if sim_safe:
                    vector.wait_ge(u_sem, s + 1)
                    if s >= 1:
                        vector.wait_ge(dve_sem, s)
                vector.tensor_tensor(
                    m_sb[:], u_sb[:], a_sb[s % nbuf][:], MULT
                ).then_inc(m_sem, 1)
                vector.wait_ge(act_sem, s + 1)
                if sim_safe:
                    vector.wait_ge(m_sem, s + 1)
                vector.tensor_tensor(
                    o_sb[s % 4][:], m_sb[:], v_sb[s % 2][:], ADD
                ).then_inc(dve_sem, 1)

    nc.compile()
    return nc


def _host_prep(x, logits, a_idx, b_idx, n_cores=N_CORES):
    x = np.asarray(x)
    logits = np.asarray(logits, dtype=np.float64)
    a_idx = np.asarray(a_idx)
    b_idx = np.asarray(b_idx)
    out_dim = logits.shape[0]
    j_core = out_dim // n_cores

    p = np.exp(logits - logits.max(-1, keepdims=True))
    p /= p.sum(-1, keepdims=True)
    w0 = p[:, 8:16].sum(-1)
    wa = p[:, 2] + p[:, 3] + p[:, 6] + p[:, 7] - p[:, 8] - p[:, 9] - p[:, 12] - p[:, 13]
    wb = p[:, 4] + p[:, 5] + p[:, 6] + p[:, 7] - p[:, 8] - p[:, 9] - p[:, 10] - p[:, 11]
    wab = (p[:, 1] - p[:, 2] - p[:, 4] - 2 * p[:, 6] - p[:, 7] + p[:, 8]
           + 2 * p[:, 9] + p[:, 11] + p[:, 13] - p[:, 14])
    # coefficients for shifted inputs x' = x - 0.5
    w0s = w0 + 0.5 * wa + 0.5 * wb + 0.25 * wab
    was = wa + 0.5 * wab
    wbs = wb + 0.5 * wab

    xT16 = np.ascontiguousarray((x.T.astype(np.float32) - 0.5).astype(np.float16))

    def percol(w):  # j = s*128 + p -> [p, s]
        return w.reshape(-1, 128).T

    in_maps = []
    for c in range(n_cores):
        sl = slice(c * j_core, (c + 1) * j_core)
        idx = np.concatenate(
            [percol(a_idx[sl]), percol(b_idx[sl])], axis=1).astype(np.int32)
        coefm = np.concatenate(
            [percol(w0s[sl]), percol(was[sl]), percol(wbs[sl]), percol(wab[sl])],
            axis=1).astype(np.float32)
        meta = np.concatenate([idx, coefm.view(np.int32)], axis=1)
        in_maps.append({"xT": xT16, "meta": np.ascontiguousarray(meta)})
    return in_maps


_NC_CACHE = {}


def _get_nc():
    if "nc" not in _NC_CACHE:
        _NC_CACHE["nc"] = _build_kernel()
    return _NC_CACHE["nc"]


def kernel(x, logits, a_idx, b_idx, _trace=False, _trace_cores=None):
    nc = _get_nc()
    in_maps = _host_prep(x, logits, a_idx, b_idx)
    kwargs = {}
    if _trace:
        kwargs = {"trace": True, "trace_cores": _trace_cores or [0]}
    res = run_bass_kernel_spmd(nc, in_maps, core_ids=list(range(N_CORES)), **kwargs)
    full = np.concatenate(
        [r["out"].T.astype(np.float32) for r in res.results], axis=1)
    if _trace:
        kernel.last_exec_time_ns = res.exec_time_ns
        kernel.last_mean_exec_time_ns = res.mean_exec_time_ns
    return full
